# revision 22
# baseline (speedup 1.0000x reference)
"""Trainium2 Bass kernel for nn_DecoderLayer (B=16,S=512,D=512,H=8).

Sharding: pure data-parallel over batch. 16 batches / 8 cores = 2 per core.
Each core runs both attention blocks + output projection for its 2 batches.

v1 rewrite over the fp32r baseline:
  - bf16 datapath for x^T / q^T / k^T / v / exp-scores / h^T tiles. The PE
    rate is keyed on the moving operand dtype, so bf16 runs 1 cycle/row at
    any moving size (fp32r needed >=256); causal tiles are emitted at their
    exact sizes. Weights stay fp32r (stationary side; mixed-dtype matmul).
  - softmax denominator broadcast via gpsimd.partition_broadcast instead of
    a K=1 matmul + scalar-engine eviction (PE and Act relief).
  - software pipelining across batches: emission order interleaves the next
    batch's transposes/projections into the current batch's attention so the
    in-order PE queue always has independent work between dependency stalls.
  - all DMAs issue on the sync (SP/HWDGE) queue.
Accumulation stays fp32 in PSUM end-to-end.
"""

import numpy as np
from contextlib import ExitStack

import concourse.bacc as bacc
import concourse.bass as bass
import concourse.mybir as mybir
import concourse.tile as tile
from concourse.bass_utils import run_bass_kernel_spmd
from concourse.masks import make_identity

B, S, D, H = 16, 512, 512, 8
DH = D // H              # 64
NCORES = 8
BPC = B // NCORES        # 2 batches per core
P = 128
NT = S // P              # 4 tiles along s/t/d
F32 = mybir.dt.float32
F32R = mybir.dt.float32r
BF16 = mybir.dt.bfloat16
EXP = mybir.ActivationFunctionType.Exp
MULT = mybir.AluOpType.mult
ADD = mybir.AluOpType.add
GE = mybir.AluOpType.is_ge


def _build(repeat=1):
    nc = bacc.Bacc("TRN2", target_bir_lowering=False)
    de = nc.dram_tensor("de_x", [BPC, S, D], F32R, kind="ExternalInput")
    en = nc.dram_tensor("en_x", [BPC, S, D], F32R, kind="ExternalInput")
    # weights ship pre-rounded to bf16 from the host (the PE requires both
    # matmul operands in the same 32-bit/16-bit class, and the activations
    # stream through as bf16)
    wq = nc.dram_tensor("wq", [D, D], BF16, kind="ExternalInput")
    wk = nc.dram_tensor("wk", [D, D], BF16, kind="ExternalInput")
    wv = nc.dram_tensor("wv", [D, D], BF16, kind="ExternalInput")
    w2 = nc.dram_tensor("w2", [D, D], BF16, kind="ExternalInput")
    b2 = nc.dram_tensor("b2", [1, D], F32, kind="ExternalInput")
    out = nc.dram_tensor("out", [BPC, S, D], F32, kind="ExternalOutput")

    with tile.TileContext(nc) as tc:
        with ExitStack() as ctx:
            _emit(ctx, tc, nc, de, en, wq, wk, wv, w2, b2, out, repeat)
    nc.finalize()
    return nc


def _emit(ctx, tc, nc, de, en, wq, wk, wv, w2, b2, out, repeat=1):
    const = ctx.enter_context(tc.tile_pool(name="const", bufs=1))
    xtp = ctx.enter_context(tc.tile_pool(name="xtp", bufs=2))
    natp = ctx.enter_context(tc.tile_pool(name="natp", bufs=8))
    qkp = ctx.enter_context(tc.tile_pool(name="qkp", bufs=2))
    vsp = ctx.enter_context(tc.tile_pool(name="vsp", bufs=2))
    htp = ctx.enter_context(tc.tile_pool(name="htp", bufs=2))
    etp = ctx.enter_context(tc.tile_pool(name="etp", bufs=12))
    rqp = ctx.enter_context(tc.tile_pool(name="rqp", bufs=4))
    prbp = ctx.enter_context(tc.tile_pool(name="prbp", bufs=4))
    stgp = ctx.enter_context(tc.tile_pool(name="stgp", bufs=4))
    outp = ctx.enter_context(tc.tile_pool(name="outp", bufs=3))
    ps = ctx.enter_context(tc.tile_pool(name="ps", bufs=2, space="PSUM"))
    ppp = ctx.enter_context(tc.tile_pool(name="ppp", bufs=3, space="PSUM"))
    ptp = ctx.enter_context(tc.tile_pool(name="ptp", bufs=1, space="PSUM"))
    pap = ctx.enter_context(tc.tile_pool(name="pap", bufs=2, space="PSUM"))

    # --- one-time constants ---
    # Memset can't write fp32r directly (invalid ISA value type), so consts
    # are built in an fp32 scratch and rounded via DVE copies.
    scr = const.tile([P, P], F32, tag="scr", name="scr")
    ident = const.tile([P, P], F32R, tag="ident", name="ident")
    make_identity(nc, scr)
    nc.vector.tensor_copy(ident, scr)
    ones_bf = const.tile([P, H], BF16, tag="ones_bf", name="ones_bf")
    nc.gpsimd.memset(scr[:, 0:H], 1.0)
    nc.vector.tensor_copy(ones_bf, scr[:, 0:H])

    b2row = const.tile([1, D], F32, tag="b2row", name="b2row")
    nc.sync.dma_start(b2row, b2[0:1, :])
    b2t = const.tile([P, D], F32, tag="b2t", name="b2t")
    nc.gpsimd.partition_broadcast(b2t, b2row[0:1, :])  # input at partition 0

    # weight loads go on the scalar/gpsimd SWDGE queues so the sync/HWDGE
    # ring is free for the first batch's input tiles at startup, and the
    # two weight streams race in parallel (wq/wk needed first)
    w_sb = {}
    for name, dram, eng in (("wq", wq, nc.scalar), ("wv", wv, nc.gpsimd),
                            ("wk", wk, nc.scalar), ("w2", w2, nc.gpsimd)):
        tiles = []
        for dt in range(NT):
            t = const.tile([P, D], BF16, tag=f"{name}{dt}", name=f"w_{name}{dt}")
            eng.dma_start(t, dram[dt * P:(dt + 1) * P, :])
            tiles.append(t)
        w_sb[name] = tiles

    # ----- per-batch stage-group builders (each returns emission closures) -

    def prefetch(b):
        nats = {}
        def go():
            for name, dram in (("de", de), ("en", en)):
                for st in range(NT):
                    natt = natp.tile([P, D], F32R, tag="nat", name="nat")
                    nc.sync.dma_start(natt, dram[b, st * P:(st + 1) * P, :])
                    nats[(name, st)] = natt
        return go, nats

    def transpose_groups(b, name, nats, xts):
        xtbig = xtp.tile([P, NT * S], BF16, tag=f"{name}T", name=f"{name}T")
        xts[name] = [xtbig[:, dt * S:(dt + 1) * S] for dt in range(NT)]
        groups = []
        for st in range(NT):
            def go(st=st, xtbig=xtbig):
                natt = nats[(name, st)]
                pt = ptp.tile([P, S], F32R, tag="pt", name="pst")
                for dt in range(NT):
                    nc.tensor.transpose(
                        pt[:, dt * P:(dt + 1) * P],
                        natt[:, dt * P:(dt + 1) * P],
                        ident,
                    )
                nc.vector.tensor_copy(
                    xtbig.rearrange("p (dt s) -> p dt s", s=S)[:, :, st * P:(st + 1) * P],
                    pt.rearrange("p (dt c) -> p dt c", c=P),
                )
            groups.append(go)
        return groups

    def qk_groups(xts, xname, wname, tagpfx, dst):
        groups = []
        for hp in range(4):
            def go(hp=hp):
                pq = ps.tile([P, D], F32, tag="ps", name="psmm")
                for dt in range(NT):
                    nc.tensor.matmul(
                        pq,
                        (w_sb[wname][dt][:, hp * P:(hp + 1) * P]),
                        (xts[xname][dt]),
                        start=dt == 0,
                        stop=dt == NT - 1,
                    )
                t = qkp.tile([P, D], BF16, tag=f"{tagpfx}{hp}", name=f"{tagpfx}{hp}")
                nc.vector.tensor_copy(t, pq)
                dst.append(t)
            groups.append(go)
        return groups

    def v_groups(lhsT_tiles, tagpfx, dst):
        # native [t, e] values for all heads; layout [128, 8*65] with a
        # ones column per head (for the softmax denominator)
        groups = []
        for tt in range(NT):
            def go(tt=tt):
                pv = ps.tile([P, D], F32, tag="ps", name="psmm")
                for dt in range(NT):
                    nc.tensor.matmul(
                        pv,
                        (lhsT_tiles[dt][:, tt * P:(tt + 1) * P]),
                        (w_sb["wv"][dt]),
                        start=dt == 0,
                        stop=dt == NT - 1,
                    )
                t = vsp.tile([P, H * (DH + 1)], BF16, tag=f"{tagpfx}{tt}", name=f"{tagpfx}{tt}")
                dv = t.rearrange("p (h x) -> p h x", x=DH + 1)
                nc.vector.tensor_copy(
                    dv[:, :, 0:DH], pv.rearrange("p (h e) -> p h e", e=DH)
                )
                nc.vector.tensor_copy(
                    dv[:, :, DH:DH + 1],
                    ones_bf.rearrange("p (h o) -> p h o", o=1),
                )
                dst.append(t)
            groups.append(go)
        return groups

    def attn_groups(qT, kT, v_s, hT, causal):
        # Three-stage software pipeline across heads in EMISSION order (the
        # engine queues are in-order, so emission order IS execution order):
        #   go(h) emits  scores/exp/select(h),  normalize(h-2),  PV(h-1).
        # PV(h-1) then never head-of-line-blocks while its exp is still on
        # the Activation queue, and the Pool broadcast only runs two heads
        # after its reciprocal so it never blocks the next affine_selects.
        sc_pend = []   # scores emitted, PV pending
        pv_pend = []   # PV emitted, normalize pending

        def emit_scores(h):
            hp, odd = divmod(h, 2)
            off = DH * odd
            e_tiles = []
            for ti in range(NT):
                s0 = ti * P if causal else 0
                pp = ppp.tile([P, D], F32, tag="pp", name="pp")
                nc.tensor.matmul(
                    pp[:, s0:D],
                    (kT[hp][off:off + DH, ti * P:(ti + 1) * P]),
                    (qT[hp][off:off + DH, s0:D]),
                    start=True,
                    stop=True,
                )
                et = etp.tile([P, D], BF16, tag="et", name="et")
                nc.scalar.activation(et[:, s0:D], pp[:, s0:D], EXP, scale=0.125)
                if causal:
                    nc.gpsimd.affine_select(
                        out=et[:, s0:s0 + P],
                        in_=et[:, s0:s0 + P],
                        compare_op=GE,
                        fill=0.0,
                        base=0,
                        pattern=[[1, P]],
                        channel_multiplier=-1,
                    )
                e_tiles.append(et)
            sc_pend.append((h, hp, odd, e_tiles))

        def emit_pv():
            h, hp, odd, e_tiles = sc_pend.pop(0)
            pa = pap.tile([DH + 1, D], F32, tag="pa", name="pa")
            for ti in range(NT):
                s0 = ti * P if causal else 0
                nc.tensor.matmul(
                    pa[:, s0:D],
                    (v_s[ti][:, h * (DH + 1):(h + 1) * (DH + 1)]),
                    (e_tiles[ti][:, s0:D]),
                    start=ti == 0,
                    stop=True,
                    skip_group_check=ti > 0,
                )
            pv_pend.append((hp, odd, pa))

        def emit_norm():
            hp, odd, pa = pv_pend.pop(0)
            # partition_broadcast ucode only reads partition 0 on HW (and
            # DMA can't read PSUM), so: reciprocal evicts the Z row to SBUF
            # (lane-locked at partition 64), a 2KB DMA hops it to partition
            # 0, then the Pool broadcast fans it out.
            rq = rqp.tile([DH + 1, D], F32, tag="rq", name="rq")
            nc.vector.reciprocal(rq[DH:DH + 1, :], pa[DH:DH + 1, :])
            rz = rqp.tile([1, D], F32, tag="rz", name="rz")
            nc.sync.dma_start(rz[0:1, :], rq[DH:DH + 1, :])
            prb = prbp.tile([DH, D], F32, tag="prb", name="prb")
            nc.gpsimd.partition_broadcast(prb, rz[0:1, :])
            if not odd:
                nc.vector.tensor_tensor(hT[hp][0:DH, :], pa[0:DH, :], prb, MULT)
            else:
                stg = stgp.tile([DH, D], BF16, tag="stg", name="stg")
                nc.vector.tensor_tensor(stg, pa[0:DH, :], prb, MULT)
                # partition shift (rows 0-63 -> 64-127) via SBUF->SBUF DMA
                nc.sync.dma_start(hT[hp][DH:P, :], stg)

        groups = []
        for h in (1, 3, 5, 7, 0, 2, 4, 6):
            def go(h=h):
                emit_scores(h)
                if pv_pend:
                    emit_norm()
                if len(sc_pend) > 1:
                    emit_pv()
            groups.append(go)

        def flush():
            while sc_pend or pv_pend:
                if sc_pend:
                    emit_pv()
                if pv_pend:
                    emit_norm()
        groups.append(flush)
        return groups

    def out_groups(b, hT):
        groups = []
        for st in range(NT):
            def go(st=st):
                po = ps.tile([P, D], F32, tag="ps", name="psmm")
                for dt in range(NT):
                    nc.tensor.matmul(
                        po,
                        (hT[dt][:, st * P:(st + 1) * P]),
                        (w_sb["w2"][dt]),
                        start=dt == 0,
                        stop=dt == NT - 1,
                    )
                ot = outp.tile([P, D], F32, tag="ot", name="ot")
                nc.vector.tensor_tensor(ot, po, b2t, ADD)
                nc.sync.dma_start(out[b, st * P:(st + 1) * P, :], ot)
            groups.append(go)
        return groups

    # ----- build the global pipeline -----

    class Batch:
        def __init__(self, b):
            self.b = b
            self.xts = {}
            self.q1, self.k1, self.v1 = [], [], []
            self.q2, self.k2, self.v2 = [], [], []
            self.pre, self.nats = prefetch(b)
            self.h1T = None
            self.h2T = None

        def early(self):
            """A_de + B(q1,k1,v1) + A_en groups, to emit before attn1."""
            a_de = transpose_groups(self.b, "de", self.nats, self.xts)
            a_en = transpose_groups(self.b, "en", self.nats, self.xts)
            bq = qk_groups(self.xts, "de", "wq", "q1T", self.q1)
            bk = qk_groups(self.xts, "de", "wk", "k1T", self.k1)
            bv = v_groups(self.xts["de"], "v1s", self.v1)
            # interleave the 12 projection groups with the 4 en-transposes
            bb = bq + bk + bv
            merged = []
            for j, g in enumerate(bb):
                merged.append(g)
                if j % 3 == 2:
                    merged.append(a_en[j // 3])
            return a_de + merged

        def attn1(self):
            self.h1T = [htp.tile([P, S], BF16, tag=f"h1T{dt}", name=f"h1T{dt}")
                        for dt in range(NT)]
            return attn_groups(self.q1, self.k1, self.v1, self.h1T, causal=True)

        def proj2(self):
            return (qk_groups(self.xts, "en", "wq", "q2T", self.q2)
                    + qk_groups(self.xts, "en", "wk", "k2T", self.k2))

        def v2g(self):
            return v_groups(self.h1T, "v2s", self.v2)

        def attn2(self):
            self.h2T = [htp.tile([P, S], BF16, tag=f"h2T{dt}", name=f"h2T{dt}")
                        for dt in range(NT)]
            return attn_groups(self.q2, self.k2, self.v2, self.h2T, causal=False)

        def outg(self):
            return out_groups(self.b, self.h2T)

    N = BPC * repeat
    batches = [Batch(bb) for _ in range(repeat) for bb in range(BPC)]

    # startup: batch 0 inputs + early stage emitted plain
    batches[0].pre()
    for g in batches[0].early():
        g()

    for n, bt in enumerate(batches):
        nxt = batches[n + 1] if n + 1 < N else None
        # attn1 interleaved with q2/k2 projections; hold the last two
        # projection groups back to cover the final heads' normalize latency
        c = bt.attn1()
        d = bt.proj2()
        di = 0
        for j, g in enumerate(c):
            g()
            if j >= 1 and di < len(d) - 2:
                d[di]()
                di += 1
        for g in d[di:]:
            g()
        # v2 projection; prefetch next batch's inputs behind it
        for g in bt.v2g():
            g()
        if nxt is not None:
            nxt.pre()
        # attn2 interleaved with the next batch's transposes + projections
        tail = nxt.early() if nxt is not None else []
        f = bt.attn2()
        ti_ = 0
        for j, g in enumerate(f):
            g()
            take = 2 if j >= 1 else 0
            for _ in range(take):
                if ti_ < len(tail):
                    tail[ti_]()
                    ti_ += 1
        for g in bt.outg():
            g()
            if ti_ < len(tail):
                tail[ti_]()
                ti_ += 1
        while ti_ < len(tail):
            tail[ti_]()
            ti_ += 1


def make_in_maps(inputs):
    import ml_dtypes
    bf16 = ml_dtypes.bfloat16
    de_x = np.asarray(inputs["de_x"], dtype=np.float32)
    en_x = np.asarray(inputs["en_x"], dtype=np.float32)
    # weights [H, D, DH] -> flat [D, H*DH], pre-rounded to bf16 on the host
    wqf = np.ascontiguousarray(np.transpose(np.asarray(inputs["Wq"], np.float32), (1, 0, 2)).reshape(D, D).astype(bf16))
    wkf = np.ascontiguousarray(np.transpose(np.asarray(inputs["Wk"], np.float32), (1, 0, 2)).reshape(D, D).astype(bf16))
    wvf = np.ascontiguousarray(np.transpose(np.asarray(inputs["Wv"], np.float32), (1, 0, 2)).reshape(D, D).astype(bf16))
    w2f = np.ascontiguousarray(np.asarray(inputs["W2"], np.float32).astype(bf16))
    b2f = np.ascontiguousarray(np.asarray(inputs["b2"], np.float32).reshape(1, D))
    in_maps = []
    for c in range(NCORES):
        in_maps.append({
            "de_x": np.ascontiguousarray(de_x[c * BPC:(c + 1) * BPC]),
            "en_x": np.ascontiguousarray(en_x[c * BPC:(c + 1) * BPC]),
            "wq": wqf, "wk": wkf, "wv": wvf, "w2": w2f, "b2": b2f,
        })
    return in_maps


def kernel(de_x, en_x, mask, Wq, Wk, Wv, W2, b2, _trace=False):
    nc = _build()
    in_maps = make_in_maps(dict(de_x=de_x, en_x=en_x, Wq=Wq, Wk=Wk, Wv=Wv,
                                W2=W2, b2=b2))
    res = run_bass_kernel_spmd(nc, in_maps, list(range(NCORES)), trace=_trace)
    outs = np.concatenate([res.results[c]["out"] for c in range(NCORES)], axis=0)
    if _trace:
        return outs, res
    return outs


# revision 24
# speedup vs baseline: 1.7499x; 1.7499x over previous
"""Trainium2 Bass kernel for nn_DecoderLayer (B=16,S=512,D=512,H=8).

Sharding: pure data-parallel over batch. 16 batches / 8 cores = 2 per core.
Each core runs both attention blocks + output projection for its 2 batches.

v1 rewrite over the fp32r baseline:
  - bf16 datapath for x^T / q^T / k^T / v / exp-scores / h^T tiles. The PE
    rate is keyed on the moving operand dtype, so bf16 runs 1 cycle/row at
    any moving size (fp32r needed >=256); causal tiles are emitted at their
    exact sizes. Weights stay fp32r (stationary side; mixed-dtype matmul).
  - softmax denominator broadcast via gpsimd.partition_broadcast instead of
    a K=1 matmul + scalar-engine eviction (PE and Act relief).
  - software pipelining across batches: emission order interleaves the next
    batch's transposes/projections into the current batch's attention so the
    in-order PE queue always has independent work between dependency stalls.
  - all DMAs issue on the sync (SP/HWDGE) queue.
Accumulation stays fp32 in PSUM end-to-end.
"""

import numpy as np
from contextlib import ExitStack

import concourse.bacc as bacc
import concourse.bass as bass
import concourse.mybir as mybir
import concourse.tile as tile
from concourse.bass_utils import run_bass_kernel_spmd
from concourse.masks import make_identity

B, S, D, H = 16, 512, 512, 8
DH = D // H              # 64
NCORES = 8
BPC = B // NCORES        # 2 batches per core
P = 128
NT = S // P              # 4 tiles along s/t/d
F32 = mybir.dt.float32
F32R = mybir.dt.float32r
BF16 = mybir.dt.bfloat16
EXP = mybir.ActivationFunctionType.Exp
MULT = mybir.AluOpType.mult
ADD = mybir.AluOpType.add
GE = mybir.AluOpType.is_ge


def _build(repeat=1):
    nc = bacc.Bacc("TRN2", target_bir_lowering=False)
    de = nc.dram_tensor("de_x", [BPC, S, D], F32R, kind="ExternalInput")
    en = nc.dram_tensor("en_x", [BPC, S, D], F32R, kind="ExternalInput")
    # weights ship pre-rounded to bf16 from the host (the PE requires both
    # matmul operands in the same 32-bit/16-bit class, and the activations
    # stream through as bf16)
    wq = nc.dram_tensor("wq", [D, D], BF16, kind="ExternalInput")
    wk = nc.dram_tensor("wk", [D, D], BF16, kind="ExternalInput")
    wv = nc.dram_tensor("wv", [D, D], BF16, kind="ExternalInput")
    w2 = nc.dram_tensor("w2", [D, D], BF16, kind="ExternalInput")
    b2 = nc.dram_tensor("b2", [1, D], F32, kind="ExternalInput")
    out = nc.dram_tensor("out", [BPC, S, D], F32, kind="ExternalOutput")

    with tile.TileContext(nc) as tc:
        with ExitStack() as ctx:
            _emit(ctx, tc, nc, de, en, wq, wk, wv, w2, b2, out, repeat)
    nc.finalize()
    return nc


def _emit(ctx, tc, nc, de, en, wq, wk, wv, w2, b2, out, repeat=1):
    const = ctx.enter_context(tc.tile_pool(name="const", bufs=1))
    xtp = ctx.enter_context(tc.tile_pool(name="xtp", bufs=2))
    natp = ctx.enter_context(tc.tile_pool(name="natp", bufs=8))
    qkp = ctx.enter_context(tc.tile_pool(name="qkp", bufs=2))
    vsp = ctx.enter_context(tc.tile_pool(name="vsp", bufs=2))
    htp = ctx.enter_context(tc.tile_pool(name="htp", bufs=2))
    etp = ctx.enter_context(tc.tile_pool(name="etp", bufs=12))
    rqp = ctx.enter_context(tc.tile_pool(name="rqp", bufs=4))
    prbp = ctx.enter_context(tc.tile_pool(name="prbp", bufs=4))
    stgp = ctx.enter_context(tc.tile_pool(name="stgp", bufs=4))
    outp = ctx.enter_context(tc.tile_pool(name="outp", bufs=3))
    ps = ctx.enter_context(tc.tile_pool(name="ps", bufs=2, space="PSUM"))
    ppp = ctx.enter_context(tc.tile_pool(name="ppp", bufs=3, space="PSUM"))
    ptp = ctx.enter_context(tc.tile_pool(name="ptp", bufs=1, space="PSUM"))
    pap = ctx.enter_context(tc.tile_pool(name="pap", bufs=2, space="PSUM"))

    # --- one-time constants ---
    # Memset can't write fp32r directly (invalid ISA value type), so consts
    # are built in an fp32 scratch and rounded via DVE copies.
    scr = const.tile([P, P], F32, tag="scr", name="scr")
    ident = const.tile([P, P], F32R, tag="ident", name="ident")
    make_identity(nc, scr)
    nc.vector.tensor_copy(ident, scr)
    ones_bf = const.tile([P, H], BF16, tag="ones_bf", name="ones_bf")
    nc.gpsimd.memset(scr[:, 0:H], 1.0)
    nc.vector.tensor_copy(ones_bf, scr[:, 0:H])

    b2row = const.tile([1, D], F32, tag="b2row", name="b2row")
    nc.sync.dma_start(b2row, b2[0:1, :])
    b2t = const.tile([P, D], F32, tag="b2t", name="b2t")
    nc.gpsimd.partition_broadcast(b2t, b2row[0:1, :])  # input at partition 0

    # weight loads go on the scalar/gpsimd SWDGE queues so the sync/HWDGE
    # ring is free for the first batch's input tiles at startup, and the
    # two weight streams race in parallel (wq/wk needed first)
    w_sb = {}
    for name, dram, eng in (("wq", wq, nc.scalar), ("wv", wv, nc.gpsimd),
                            ("wk", wk, nc.scalar), ("w2", w2, nc.gpsimd)):
        tiles = []
        for dt in range(NT):
            t = const.tile([P, D], BF16, tag=f"{name}{dt}", name=f"w_{name}{dt}")
            eng.dma_start(t, dram[dt * P:(dt + 1) * P, :])
            tiles.append(t)
        w_sb[name] = tiles

    # ----- per-batch stage-group builders (each returns emission closures) -

    def prefetch(b):
        nats = {}
        def go():
            for name, dram in (("de", de), ("en", en)):
                for st in range(NT):
                    natt = natp.tile([P, D], F32R, tag="nat", name="nat")
                    nc.sync.dma_start(natt, dram[b, st * P:(st + 1) * P, :])
                    nats[(name, st)] = natt
        return go, nats

    def transpose_groups(b, name, nats, xts):
        xtbig = xtp.tile([P, NT * S], BF16, tag=f"{name}T", name=f"{name}T")
        xts[name] = [xtbig[:, dt * S:(dt + 1) * S] for dt in range(NT)]
        groups = []
        for st in range(NT):
            def go(st=st, xtbig=xtbig):
                natt = nats[(name, st)]
                pt = ptp.tile([P, S], F32R, tag="pt", name="pst")
                for dt in range(NT):
                    nc.tensor.transpose(
                        pt[:, dt * P:(dt + 1) * P],
                        natt[:, dt * P:(dt + 1) * P],
                        ident,
                    )
                nc.vector.tensor_copy(
                    xtbig.rearrange("p (dt s) -> p dt s", s=S)[:, :, st * P:(st + 1) * P],
                    pt.rearrange("p (dt c) -> p dt c", c=P),
                )
            groups.append(go)
        return groups

    def qk_groups(xts, xname, wname, tagpfx, dst):
        groups = []
        for hp in range(4):
            def go(hp=hp):
                pq = ps.tile([P, D], F32, tag="ps", name="psmm")
                for dt in range(NT):
                    nc.tensor.matmul(
                        pq,
                        (w_sb[wname][dt][:, hp * P:(hp + 1) * P]),
                        (xts[xname][dt]),
                        start=dt == 0,
                        stop=dt == NT - 1,
                    )
                t = qkp.tile([P, D], BF16, tag=f"{tagpfx}{hp}", name=f"{tagpfx}{hp}")
                nc.vector.tensor_copy(t, pq)
                dst.append(t)
            groups.append(go)
        return groups

    def v_groups(lhsT_tiles, tagpfx, dst):
        # native [t, e] values for all heads; layout [128, 8*65] with a
        # ones column per head (for the softmax denominator)
        groups = []
        for tt in range(NT):
            def go(tt=tt):
                pv = ps.tile([P, D], F32, tag="ps", name="psmm")
                for dt in range(NT):
                    nc.tensor.matmul(
                        pv,
                        (lhsT_tiles[dt][:, tt * P:(tt + 1) * P]),
                        (w_sb["wv"][dt]),
                        start=dt == 0,
                        stop=dt == NT - 1,
                    )
                t = vsp.tile([P, H * (DH + 1)], BF16, tag=f"{tagpfx}{tt}", name=f"{tagpfx}{tt}")
                dv = t.rearrange("p (h x) -> p h x", x=DH + 1)
                nc.vector.tensor_copy(
                    dv[:, :, 0:DH], pv.rearrange("p (h e) -> p h e", e=DH)
                )
                nc.vector.tensor_copy(
                    dv[:, :, DH:DH + 1],
                    ones_bf.rearrange("p (h o) -> p h o", o=1),
                )
                dst.append(t)
            groups.append(go)
        return groups

    def attn_groups(qT, kT, v_s, hT, causal):
        # Three-stage software pipeline across heads in EMISSION order (the
        # engine queues are in-order, so emission order IS execution order):
        #   go(h) emits  scores/exp/select(h),  normalize(h-2),  PV(h-1).
        # PV(h-1) then never head-of-line-blocks while its exp is still on
        # the Activation queue, and the Pool broadcast only runs two heads
        # after its reciprocal so it never blocks the next affine_selects.
        sc_pend = []   # scores emitted, PV pending
        pv_pend = []   # PV emitted, normalize pending

        def emit_scores(h):
            hp, odd = divmod(h, 2)
            off = DH * odd
            e_tiles = []
            for ti in range(NT):
                s0 = ti * P if causal else 0
                pp = ppp.tile([P, D], F32, tag="pp", name="pp")
                nc.tensor.matmul(
                    pp[:, s0:D],
                    (kT[hp][off:off + DH, ti * P:(ti + 1) * P]),
                    (qT[hp][off:off + DH, s0:D]),
                    start=True,
                    stop=True,
                )
                et = etp.tile([P, D], BF16, tag="et", name="et")
                nc.scalar.activation(et[:, s0:D], pp[:, s0:D], EXP, scale=0.125)
                if causal:
                    nc.gpsimd.affine_select(
                        out=et[:, s0:s0 + P],
                        in_=et[:, s0:s0 + P],
                        compare_op=GE,
                        fill=0.0,
                        base=0,
                        pattern=[[1, P]],
                        channel_multiplier=-1,
                    )
                e_tiles.append(et)
            sc_pend.append((h, hp, odd, e_tiles))

        def emit_pv():
            h, hp, odd, e_tiles = sc_pend.pop(0)
            pa = pap.tile([DH + 1, D], F32, tag="pa", name="pa")
            for ti in range(NT):
                s0 = ti * P if causal else 0
                nc.tensor.matmul(
                    pa[:, s0:D],
                    (v_s[ti][:, h * (DH + 1):(h + 1) * (DH + 1)]),
                    (e_tiles[ti][:, s0:D]),
                    start=ti == 0,
                    stop=True,
                    skip_group_check=ti > 0,
                )
            pv_pend.append((hp, odd, pa))

        def emit_norm():
            hp, odd, pa = pv_pend.pop(0)
            # partition_broadcast ucode only reads partition 0 on HW (and DMA
            # can't read PSUM), so: Act evicts the Z row to SBUF (lane-locked
            # at partition 64), a 2KB DMA hops it to partition 0, the approx
            # reciprocal (~5x faster than the iterative-divide reciprocal(),
            # ~18 correct bits, SBUF-only — it misreads PSUM) inverts it, and
            # the Pool broadcast fans it out. Z >= 1 avoids the approx's
            # undefined edge cases.
            rq = rqp.tile([DH + 1, D], F32, tag="rq", name="rq")
            nc.scalar.copy(rq[DH:DH + 1, :], pa[DH:DH + 1, :])
            rz = rqp.tile([1, 2 * D], F32, tag="rz", name="rz")
            nc.sync.dma_start(rz[0:1, 0:D], rq[DH:DH + 1, :])
            nc.vector.reciprocal_approx_fast(rz[0:1, D:2 * D], rz[0:1, 0:D])
            prb = prbp.tile([DH, D], F32, tag="prb", name="prb")
            nc.gpsimd.partition_broadcast(prb, rz[0:1, D:2 * D])
            if not odd:
                nc.vector.tensor_tensor(hT[hp][0:DH, :], pa[0:DH, :], prb, MULT)
            else:
                stg = stgp.tile([DH, D], BF16, tag="stg", name="stg")
                nc.vector.tensor_tensor(stg, pa[0:DH, :], prb, MULT)
                # partition shift (rows 0-63 -> 64-127) via SBUF->SBUF DMA
                nc.sync.dma_start(hT[hp][DH:P, :], stg)

        groups = []
        for h in (1, 3, 5, 7, 0, 2, 4, 6):
            def go(h=h):
                emit_scores(h)
                if pv_pend:
                    emit_norm()
                if len(sc_pend) > 1:
                    emit_pv()
            groups.append(go)

        def flush():
            while sc_pend or pv_pend:
                if sc_pend:
                    emit_pv()
                if pv_pend:
                    emit_norm()
        groups.append(flush)
        return groups

    def out_groups(b, hT):
        groups = []
        for st in range(NT):
            def go(st=st):
                po = ps.tile([P, D], F32, tag="ps", name="psmm")
                for dt in range(NT):
                    nc.tensor.matmul(
                        po,
                        (hT[dt][:, st * P:(st + 1) * P]),
                        (w_sb["w2"][dt]),
                        start=dt == 0,
                        stop=dt == NT - 1,
                    )
                ot = outp.tile([P, D], F32, tag="ot", name="ot")
                nc.vector.tensor_tensor(ot, po, b2t, ADD)
                nc.sync.dma_start(out[b, st * P:(st + 1) * P, :], ot)
            groups.append(go)
        return groups

    # ----- build the global pipeline -----

    class Batch:
        def __init__(self, b):
            self.b = b
            self.xts = {}
            self.q1, self.k1, self.v1 = [], [], []
            self.q2, self.k2, self.v2 = [], [], []
            self.pre, self.nats = prefetch(b)
            self.h1T = None
            self.h2T = None

        def early(self):
            """A_de + B(q1,k1,v1) + A_en groups, to emit before attn1."""
            a_de = transpose_groups(self.b, "de", self.nats, self.xts)
            a_en = transpose_groups(self.b, "en", self.nats, self.xts)
            bq = qk_groups(self.xts, "de", "wq", "q1T", self.q1)
            bk = qk_groups(self.xts, "de", "wk", "k1T", self.k1)
            bv = v_groups(self.xts["de"], "v1s", self.v1)
            # interleave the 12 projection groups with the 4 en-transposes
            bb = bq + bk + bv
            merged = []
            for j, g in enumerate(bb):
                merged.append(g)
                if j % 3 == 2:
                    merged.append(a_en[j // 3])
            return a_de + merged

        def attn1(self):
            self.h1T = [htp.tile([P, S], BF16, tag=f"h1T{dt}", name=f"h1T{dt}")
                        for dt in range(NT)]
            return attn_groups(self.q1, self.k1, self.v1, self.h1T, causal=True)

        def proj2(self):
            return (qk_groups(self.xts, "en", "wq", "q2T", self.q2)
                    + qk_groups(self.xts, "en", "wk", "k2T", self.k2))

        def v2g(self):
            return v_groups(self.h1T, "v2s", self.v2)

        def attn2(self):
            self.h2T = [htp.tile([P, S], BF16, tag=f"h2T{dt}", name=f"h2T{dt}")
                        for dt in range(NT)]
            return attn_groups(self.q2, self.k2, self.v2, self.h2T, causal=False)

        def outg(self):
            return out_groups(self.b, self.h2T)

    N = BPC * repeat
    batches = [Batch(bb) for _ in range(repeat) for bb in range(BPC)]

    # startup: batch 0 inputs + early stage emitted plain
    batches[0].pre()
    for g in batches[0].early():
        g()

    for n, bt in enumerate(batches):
        nxt = batches[n + 1] if n + 1 < N else None
        # attn1 interleaved with q2/k2 projections; hold the last two
        # projection groups back to cover the final heads' normalize latency
        c = bt.attn1()
        d = bt.proj2()
        di = 0
        for j, g in enumerate(c):
            g()
            if j >= 1 and di < len(d) - 2:
                d[di]()
                di += 1
        for g in d[di:]:
            g()
        # v2 projection; prefetch next batch's inputs behind it
        for g in bt.v2g():
            g()
        if nxt is not None:
            nxt.pre()
        # attn2 interleaved with the next batch's transposes + projections
        tail = nxt.early() if nxt is not None else []
        f = bt.attn2()
        ti_ = 0
        for j, g in enumerate(f):
            g()
            take = 2 if j >= 1 else 0
            for _ in range(take):
                if ti_ < len(tail):
                    tail[ti_]()
                    ti_ += 1
        for g in bt.outg():
            g()
            if ti_ < len(tail):
                tail[ti_]()
                ti_ += 1
        while ti_ < len(tail):
            tail[ti_]()
            ti_ += 1


def make_in_maps(inputs):
    import ml_dtypes
    bf16 = ml_dtypes.bfloat16
    de_x = np.asarray(inputs["de_x"], dtype=np.float32)
    en_x = np.asarray(inputs["en_x"], dtype=np.float32)
    # weights [H, D, DH] -> flat [D, H*DH], pre-rounded to bf16 on the host
    wqf = np.ascontiguousarray(np.transpose(np.asarray(inputs["Wq"], np.float32), (1, 0, 2)).reshape(D, D).astype(bf16))
    wkf = np.ascontiguousarray(np.transpose(np.asarray(inputs["Wk"], np.float32), (1, 0, 2)).reshape(D, D).astype(bf16))
    wvf = np.ascontiguousarray(np.transpose(np.asarray(inputs["Wv"], np.float32), (1, 0, 2)).reshape(D, D).astype(bf16))
    w2f = np.ascontiguousarray(np.asarray(inputs["W2"], np.float32).astype(bf16))
    b2f = np.ascontiguousarray(np.asarray(inputs["b2"], np.float32).reshape(1, D))
    in_maps = []
    for c in range(NCORES):
        in_maps.append({
            "de_x": np.ascontiguousarray(de_x[c * BPC:(c + 1) * BPC]),
            "en_x": np.ascontiguousarray(en_x[c * BPC:(c + 1) * BPC]),
            "wq": wqf, "wk": wkf, "wv": wvf, "w2": w2f, "b2": b2f,
        })
    return in_maps


def kernel(de_x, en_x, mask, Wq, Wk, Wv, W2, b2, _trace=False):
    nc = _build()
    in_maps = make_in_maps(dict(de_x=de_x, en_x=en_x, Wq=Wq, Wk=Wk, Wv=Wv,
                                W2=W2, b2=b2))
    res = run_bass_kernel_spmd(nc, in_maps, list(range(NCORES)), trace=_trace)
    outs = np.concatenate([res.results[c]["out"] for c in range(NCORES)], axis=0)
    if _trace:
        return outs, res
    return outs


# revision 28
# speedup vs baseline: 2.8372x; 1.6214x over previous
"""Trainium2 Bass kernel for nn_DecoderLayer (B=16,S=512,D=512,H=8).

Sharding: pure data-parallel over batch. 16 batches / 8 cores = 2 per core.
Each core runs both attention blocks + output projection for its 2 batches.

v1 rewrite over the fp32r baseline:
  - bf16 datapath for x^T / q^T / k^T / v / exp-scores / h^T tiles. The PE
    rate is keyed on the moving operand dtype, so bf16 runs 1 cycle/row at
    any moving size (fp32r needed >=256); causal tiles are emitted at their
    exact sizes. Weights stay fp32r (stationary side; mixed-dtype matmul).
  - softmax denominator broadcast via gpsimd.partition_broadcast instead of
    a K=1 matmul + scalar-engine eviction (PE and Act relief).
  - software pipelining across batches: emission order interleaves the next
    batch's transposes/projections into the current batch's attention so the
    in-order PE queue always has independent work between dependency stalls.
  - all DMAs issue on the sync (SP/HWDGE) queue.
Accumulation stays fp32 in PSUM end-to-end.
"""

import numpy as np
from contextlib import ExitStack

import concourse.bacc as bacc
import concourse.bass as bass
import concourse.mybir as mybir
import concourse.tile as tile
from concourse.bass_utils import run_bass_kernel_spmd
from concourse.masks import make_identity

B, S, D, H = 16, 512, 512, 8
DH = D // H              # 64
NCORES = 8
BPC = B // NCORES        # 2 batches per core
P = 128
NT = S // P              # 4 tiles along s/t/d
F32 = mybir.dt.float32
F32R = mybir.dt.float32r
BF16 = mybir.dt.bfloat16
EXP = mybir.ActivationFunctionType.Exp
MULT = mybir.AluOpType.mult
ADD = mybir.AluOpType.add
GE = mybir.AluOpType.is_ge


def _build(repeat=1):
    nc = bacc.Bacc("TRN2", target_bir_lowering=False)
    de = nc.dram_tensor("de_x", [BPC, S, D], F32, kind="ExternalInput")
    en = nc.dram_tensor("en_x", [BPC, S, D], F32, kind="ExternalInput")
    # weights ship pre-rounded to bf16 from the host (the PE requires both
    # matmul operands in the same 32-bit/16-bit class, and the activations
    # stream through as bf16)
    wq = nc.dram_tensor("wq", [D, D], BF16, kind="ExternalInput")
    wk = nc.dram_tensor("wk", [D, D], BF16, kind="ExternalInput")
    wv = nc.dram_tensor("wv", [D, D], BF16, kind="ExternalInput")
    w2 = nc.dram_tensor("w2", [D, D], BF16, kind="ExternalInput")
    b2 = nc.dram_tensor("b2", [1, D], F32, kind="ExternalInput")
    out = nc.dram_tensor("out", [BPC, S, D], F32, kind="ExternalOutput")

    with tile.TileContext(nc) as tc:
        with ExitStack() as ctx:
            _emit(ctx, tc, nc, de, en, wq, wk, wv, w2, b2, out, repeat)
    nc.finalize()
    return nc


def _emit(ctx, tc, nc, de, en, wq, wk, wv, w2, b2, out, repeat=1):
    const = ctx.enter_context(tc.tile_pool(name="const", bufs=1))
    xtp = ctx.enter_context(tc.tile_pool(name="xtp", bufs=2))
    natp = ctx.enter_context(tc.tile_pool(name="natp", bufs=8))
    qkp = ctx.enter_context(tc.tile_pool(name="qkp", bufs=2))
    vsp = ctx.enter_context(tc.tile_pool(name="vsp", bufs=2))
    htp = ctx.enter_context(tc.tile_pool(name="htp", bufs=2))
    etp = ctx.enter_context(tc.tile_pool(name="etp", bufs=12))
    rqp = ctx.enter_context(tc.tile_pool(name="rqp", bufs=4))
    prbp = ctx.enter_context(tc.tile_pool(name="prbp", bufs=4))
    stgp = ctx.enter_context(tc.tile_pool(name="stgp", bufs=4))
    outp = ctx.enter_context(tc.tile_pool(name="outp", bufs=3))
    ps = ctx.enter_context(tc.tile_pool(name="ps", bufs=2, space="PSUM"))
    ppp = ctx.enter_context(tc.tile_pool(name="ppp", bufs=3, space="PSUM"))
    ptp = ctx.enter_context(tc.tile_pool(name="ptp", bufs=1, space="PSUM"))
    pap = ctx.enter_context(tc.tile_pool(name="pap", bufs=2, space="PSUM"))

    # --- one-time constants ---
    # Memset can't write fp32r directly (invalid ISA value type), so consts
    # are built in an fp32 scratch and rounded via DVE copies.
    scr = const.tile([P, P], F32, tag="scr", name="scr")
    ident_bf = const.tile([P, P], BF16, tag="ident_bf", name="ident_bf")
    make_identity(nc, scr)
    nc.vector.tensor_copy(ident_bf, scr)
    ones_bf = const.tile([P, H], BF16, tag="ones_bf", name="ones_bf")
    nc.gpsimd.memset(scr[:, 0:H], 1.0)
    nc.vector.tensor_copy(ones_bf, scr[:, 0:H])

    b2row = const.tile([1, D], F32, tag="b2row", name="b2row")
    nc.sync.dma_start(b2row, b2[0:1, :])
    b2t = const.tile([P, D], F32, tag="b2t", name="b2t")
    nc.gpsimd.partition_broadcast(b2t, b2row[0:1, :])  # input at partition 0

    # weight loads go on the scalar/gpsimd SWDGE queues so the sync/HWDGE
    # ring is free for the first batch's input tiles at startup, and the
    # two weight streams race in parallel (wq/wk needed first)
    w_sb = {}
    for name, dram, eng in (("wq", wq, nc.scalar), ("wv", wv, nc.gpsimd),
                            ("wk", wk, nc.scalar), ("w2", w2, nc.gpsimd)):
        tiles = []
        for dt in range(NT):
            t = const.tile([P, D], BF16, tag=f"{name}{dt}", name=f"w_{name}{dt}")
            eng.dma_start(t, dram[dt * P:(dt + 1) * P, :])
            tiles.append(t)
        w_sb[name] = tiles

    # ----- per-batch stage-group builders (each returns emission closures) -

    def prefetch(b):
        nats = {}
        def go():
            for name, dram in (("de", de), ("en", en)):
                for st in range(NT):
                    natt = natp.tile([P, D], F32, tag="nat", name="nat")
                    nc.sync.dma_start(natt, dram[b, st * P:(st + 1) * P, :])
                    nats[(name, st)] = natt
        return go, nats

    def transpose_groups(b, name, nats, xts):
        xtbig = xtp.tile([P, NT * S], BF16, tag=f"{name}T", name=f"{name}T")
        xts[name] = [xtbig[:, dt * S:(dt + 1) * S] for dt in range(NT)]
        groups = []
        for st in range(NT):
            def go(st=st, xtbig=xtbig):
                natt = nats[(name, st)]
                # convert to bf16 on the Act queue first: the PE transpose
                # then runs at 1 cyc/row (vs 1.5 fp32r) and the DVE eviction
                # gets the 2-byte 2x mode
                natb = natp.tile([P, D], BF16, tag="natb", name="natb")
                nc.scalar.copy(natb, natt)
                pt = ptp.tile([P, S], BF16, tag="pt", name="pst")
                for dt in range(NT):
                    nc.tensor.transpose(
                        pt[:, dt * P:(dt + 1) * P],
                        natb[:, dt * P:(dt + 1) * P],
                        ident_bf,
                    )
                nc.vector.tensor_copy(
                    xtbig.rearrange("p (dt s) -> p dt s", s=S)[:, :, st * P:(st + 1) * P],
                    pt.rearrange("p (dt c) -> p dt c", c=P),
                )
            groups.append(go)
        return groups

    def qk_groups(xts, xname, wname, tagpfx, dst):
        groups = []
        for hp in range(4):
            def go(hp=hp):
                pq = ps.tile([P, D], F32, tag="ps", name="psmm")
                for dt in range(NT):
                    nc.tensor.matmul(
                        pq,
                        (w_sb[wname][dt][:, hp * P:(hp + 1) * P]),
                        (xts[xname][dt]),
                        start=dt == 0,
                        stop=dt == NT - 1,
                    )
                t = qkp.tile([P, D], BF16, tag=f"{tagpfx}{hp}", name=f"{tagpfx}{hp}")
                nc.vector.tensor_copy(t, pq)
                dst.append(t)
            groups.append(go)
        return groups

    def v_groups(lhsT_tiles, tagpfx, dst):
        # native [t, e] values for all heads; layout [128, 8*65] with a
        # ones column per head (for the softmax denominator)
        groups = []
        for tt in range(NT):
            def go(tt=tt):
                pv = ps.tile([P, D], F32, tag="ps", name="psmm")
                for dt in range(NT):
                    nc.tensor.matmul(
                        pv,
                        (lhsT_tiles[dt][:, tt * P:(tt + 1) * P]),
                        (w_sb["wv"][dt]),
                        start=dt == 0,
                        stop=dt == NT - 1,
                    )
                t = vsp.tile([P, H * (DH + 1)], BF16, tag=f"{tagpfx}{tt}", name=f"{tagpfx}{tt}")
                dv = t.rearrange("p (h x) -> p h x", x=DH + 1)
                nc.vector.tensor_copy(
                    dv[:, :, 0:DH], pv.rearrange("p (h e) -> p h e", e=DH)
                )
                nc.vector.tensor_copy(
                    dv[:, :, DH:DH + 1],
                    ones_bf.rearrange("p (h o) -> p h o", o=1),
                )
                dst.append(t)
            groups.append(go)
        return groups

    def attn_groups(qT, kT, v_s, hT, causal):
        # Three-stage software pipeline across heads in EMISSION order (the
        # engine queues are in-order, so emission order IS execution order):
        #   go(h) emits  scores/exp/select(h),  normalize(h-2),  PV(h-1).
        # PV(h-1) then never head-of-line-blocks while its exp is still on
        # the Activation queue, and the Pool broadcast only runs two heads
        # after its reciprocal so it never blocks the next affine_selects.
        sc_pend = []   # scores emitted, PV pending
        pv_pend = []   # PV emitted, normalize pending

        def emit_scores(h):
            hp, odd = divmod(h, 2)
            off = DH * odd
            e_tiles = []
            for ti in range(NT):
                s0 = ti * P if causal else 0
                pp = ppp.tile([P, D], F32, tag="pp", name="pp")
                nc.tensor.matmul(
                    pp[:, s0:D],
                    (kT[hp][off:off + DH, ti * P:(ti + 1) * P]),
                    (qT[hp][off:off + DH, s0:D]),
                    start=True,
                    stop=True,
                )
                et = etp.tile([P, D], BF16, tag="et", name="et")
                nc.scalar.activation(et[:, s0:D], pp[:, s0:D], EXP, scale=0.125)
                if causal:
                    nc.gpsimd.affine_select(
                        out=et[:, s0:s0 + P],
                        in_=et[:, s0:s0 + P],
                        compare_op=GE,
                        fill=0.0,
                        base=0,
                        pattern=[[1, P]],
                        channel_multiplier=-1,
                    )
                e_tiles.append(et)
            sc_pend.append((h, hp, odd, e_tiles))

        def emit_pv():
            h, hp, odd, e_tiles = sc_pend.pop(0)
            pa = pap.tile([DH + 1, D], F32, tag="pa", name="pa")
            for ti in range(NT):
                s0 = ti * P if causal else 0
                nc.tensor.matmul(
                    pa[:, s0:D],
                    (v_s[ti][:, h * (DH + 1):(h + 1) * (DH + 1)]),
                    (e_tiles[ti][:, s0:D]),
                    start=ti == 0,
                    stop=True,
                    skip_group_check=ti > 0,
                )
            pv_pend.append((hp, odd, pa))

        def emit_norm():
            hp, odd, pa = pv_pend.pop(0)
            # partition_broadcast ucode only reads partition 0 on HW (and DMA
            # can't read PSUM), so: Act evicts the Z row to SBUF (lane-locked
            # at partition 64), a 2KB DMA hops it to partition 0, the approx
            # reciprocal (~5x faster than the iterative-divide reciprocal(),
            # ~18 correct bits, SBUF-only — it misreads PSUM) inverts it, and
            # the Pool broadcast fans it out. Z >= 1 avoids the approx's
            # undefined edge cases.
            rq = rqp.tile([DH + 1, D], F32, tag="rq", name="rq")
            nc.scalar.copy(rq[DH:DH + 1, :], pa[DH:DH + 1, :])
            rz = rqp.tile([1, 2 * D], F32, tag="rz", name="rz")
            nc.sync.dma_start(rz[0:1, 0:D], rq[DH:DH + 1, :])
            nc.vector.reciprocal_approx_fast(rz[0:1, D:2 * D], rz[0:1, 0:D])
            prb = prbp.tile([DH, D], F32, tag="prb", name="prb")
            nc.gpsimd.partition_broadcast(prb, rz[0:1, D:2 * D])
            if not odd:
                nc.vector.tensor_tensor(hT[hp][0:DH, :], pa[0:DH, :], prb, MULT)
            else:
                stg = stgp.tile([DH, D], BF16, tag="stg", name="stg")
                nc.vector.tensor_tensor(stg, pa[0:DH, :], prb, MULT)
                # partition shift (rows 0-63 -> 64-127) via SBUF->SBUF DMA
                nc.sync.dma_start(hT[hp][DH:P, :], stg)

        groups = []
        for h in (1, 3, 5, 7, 0, 2, 4, 6):
            def go(h=h):
                emit_scores(h)
                if pv_pend:
                    emit_norm()
                if len(sc_pend) > 1:
                    emit_pv()
            groups.append(go)

        def flush():
            while sc_pend or pv_pend:
                if sc_pend:
                    emit_pv()
                if pv_pend:
                    emit_norm()
        groups.append(flush)
        return groups

    def out_groups(b, hT):
        groups = []
        for st in range(NT):
            def go(st=st):
                po = ps.tile([P, D], F32, tag="ps", name="psmm")
                for dt in range(NT):
                    nc.tensor.matmul(
                        po,
                        (hT[dt][:, st * P:(st + 1) * P]),
                        (w_sb["w2"][dt]),
                        start=dt == 0,
                        stop=dt == NT - 1,
                    )
                ot = outp.tile([P, D], F32, tag="ot", name="ot")
                nc.vector.tensor_tensor(ot, po, b2t, ADD)
                nc.sync.dma_start(out[b, st * P:(st + 1) * P, :], ot)
            groups.append(go)
        return groups

    # ----- build the global pipeline -----

    class Batch:
        def __init__(self, b):
            self.b = b
            self.xts = {}
            self.q1, self.k1, self.v1 = [], [], []
            self.q2, self.k2, self.v2 = [], [], []
            self.pre, self.nats = prefetch(b)
            self.h1T = None
            self.h2T = None

        def early(self):
            """A_de + B(q1,k1,v1) + A_en groups, to emit before attn1."""
            a_de = transpose_groups(self.b, "de", self.nats, self.xts)
            a_en = transpose_groups(self.b, "en", self.nats, self.xts)
            bq = qk_groups(self.xts, "de", "wq", "q1T", self.q1)
            bk = qk_groups(self.xts, "de", "wk", "k1T", self.k1)
            bv = v_groups(self.xts["de"], "v1s", self.v1)
            # interleave the 12 projection groups with the 4 en-transposes
            bb = bq + bk + bv
            merged = []
            for j, g in enumerate(bb):
                merged.append(g)
                if j % 3 == 2:
                    merged.append(a_en[j // 3])
            return a_de + merged

        def attn1(self):
            self.h1T = [htp.tile([P, S], BF16, tag=f"h1T{dt}", name=f"h1T{dt}")
                        for dt in range(NT)]
            return attn_groups(self.q1, self.k1, self.v1, self.h1T, causal=True)

        def proj2(self):
            return (qk_groups(self.xts, "en", "wq", "q2T", self.q2)
                    + qk_groups(self.xts, "en", "wk", "k2T", self.k2))

        def v2g(self):
            return v_groups(self.h1T, "v2s", self.v2)

        def attn2(self):
            self.h2T = [htp.tile([P, S], BF16, tag=f"h2T{dt}", name=f"h2T{dt}")
                        for dt in range(NT)]
            return attn_groups(self.q2, self.k2, self.v2, self.h2T, causal=False)

        def outg(self):
            return out_groups(self.b, self.h2T)

    N = BPC * repeat
    batches = [Batch(bb) for _ in range(repeat) for bb in range(BPC)]

    # startup: batch 0 inputs + early stage emitted plain
    batches[0].pre()
    for g in batches[0].early():
        g()

    for n, bt in enumerate(batches):
        nxt = batches[n + 1] if n + 1 < N else None
        # attn1 interleaved with q2/k2 projections; hold the last two
        # projection groups back to cover the final heads' normalize latency
        c = bt.attn1()
        d = bt.proj2()
        di = 0
        for j, g in enumerate(c):
            g()
            if j >= 1 and di < len(d) - 2:
                d[di]()
                di += 1
        for g in d[di:]:
            g()
        # v2 projection; prefetch next batch's inputs behind it
        for g in bt.v2g():
            g()
        if nxt is not None:
            nxt.pre()
        # attn2 interleaved with the next batch's transposes + projections
        tail = nxt.early() if nxt is not None else []
        f = bt.attn2()
        ti_ = 0
        for j, g in enumerate(f):
            g()
            take = 2 if j >= 1 else 0
            for _ in range(take):
                if ti_ < len(tail):
                    tail[ti_]()
                    ti_ += 1
        for g in bt.outg():
            g()
            if ti_ < len(tail):
                tail[ti_]()
                ti_ += 1
        while ti_ < len(tail):
            tail[ti_]()
            ti_ += 1


def make_in_maps(inputs):
    import ml_dtypes
    bf16 = ml_dtypes.bfloat16
    de_x = np.asarray(inputs["de_x"], dtype=np.float32)
    en_x = np.asarray(inputs["en_x"], dtype=np.float32)
    # weights [H, D, DH] -> flat [D, H*DH], pre-rounded to bf16 on the host
    wqf = np.ascontiguousarray(np.transpose(np.asarray(inputs["Wq"], np.float32), (1, 0, 2)).reshape(D, D).astype(bf16))
    wkf = np.ascontiguousarray(np.transpose(np.asarray(inputs["Wk"], np.float32), (1, 0, 2)).reshape(D, D).astype(bf16))
    wvf = np.ascontiguousarray(np.transpose(np.asarray(inputs["Wv"], np.float32), (1, 0, 2)).reshape(D, D).astype(bf16))
    w2f = np.ascontiguousarray(np.asarray(inputs["W2"], np.float32).astype(bf16))
    b2f = np.ascontiguousarray(np.asarray(inputs["b2"], np.float32).reshape(1, D))
    in_maps = []
    for c in range(NCORES):
        in_maps.append({
            "de_x": np.ascontiguousarray(de_x[c * BPC:(c + 1) * BPC]),
            "en_x": np.ascontiguousarray(en_x[c * BPC:(c + 1) * BPC]),
            "wq": wqf, "wk": wkf, "wv": wvf, "w2": w2f, "b2": b2f,
        })
    return in_maps


def kernel(de_x, en_x, mask, Wq, Wk, Wv, W2, b2, _trace=False):
    nc = _build()
    in_maps = make_in_maps(dict(de_x=de_x, en_x=en_x, Wq=Wq, Wk=Wk, Wv=Wv,
                                W2=W2, b2=b2))
    res = run_bass_kernel_spmd(nc, in_maps, list(range(NCORES)), trace=_trace)
    outs = np.concatenate([res.results[c]["out"] for c in range(NCORES)], axis=0)
    if _trace:
        return outs, res
    return outs
